# revision 29
# baseline (speedup 1.0000x reference)
"""Trainium2 Bass kernel for nn_DenseReluGMMConvNetwork (2-layer GMMConv GNN).

Self-contained: takes FULL inputs, shards nodes across 8 NeuronCores,
runs one SPMD Bass program (gather / GMM weights / scatter-matmul /
BN via AllReduce / inter-layer AllGather), returns FULL [50000, 64] output.

v5: - scatter one-hot matrices DMA'd as fp8 (exact for 0/1, half of bf16
      bytes); 1/deg folded into the GMM weights on device.
    - gather table bf16 padded to 128 cols (256B rows): gathered xg is bf16.
    - GMM weights computed in transposed slot-major layout [128, K, ncht]
      with host-replicated mu/inv2sigma2 coefficient tiles: 3 big contiguous
      DVE ops + 2 adds instead of many small strided ones.
    - xw multiply batched per (super-group, half): 2 DVE ops per super-group.
    - chunk-structured table layout: inter-layer AllGather runs in 4
      contiguous chunks overlapped with layer-0 compute, carrying PRE-BN h.
      BN0 is split: per-channel shift + ReLU on gathered tiles (1 DVE add +
      scalar-engine ReLU); the BN scale is folded into G1/RD1 on device
      (requires gamma>0, true for this model).
"""

import os
import sys

sys.path.insert(0, "/opt/trn_rl_repo")

import numpy as np
import ml_dtypes

BF16 = ml_dtypes.bfloat16
FP8 = ml_dtypes.float8_e4m3fn

# ---- problem constants ----
N = 50000
E = 800000
D = 3
K = 4
C = 64
NCORES = 8
EPS = 1e-15
BN_EPS = 1e-5
SG_WINDOWS = 2          # windows per gather super-group
HALF_SPLIT = 32768      # int16 index range split (= first two AG chunks)
GMAX = 32               # chunks per gather instruction
AG_CUTS = (16, 32, 44)  # AllGather chunk boundaries (window index)

LAST_RESULT = None


def _derived():
    npc = N // NCORES
    win = (npc + 127) // 128          # windows per core
    node_pad = win * 128              # padded rows per core
    trows = NCORES * node_pad         # padded gather-table rows
    return npc, win, node_pad, trows


def _ag_chunks(win):
    cuts = [0] + [c for c in AG_CUTS if c < win] + [win]
    return [(cuts[i], cuts[i + 1]) for i in range(len(cuts) - 1)]


def _prow_of(node):
    """Global gather-table row for each node id (chunk-structured layout)."""
    npc, win, node_pad, trows = _derived()
    chunks = _ag_chunks(win)
    m = node // npc
    l = node - m * npc
    w = l >> 7
    j = l & 127
    base = np.zeros_like(node)
    cbase = 0
    for (w0, w1) in chunks:
        nw = w1 - w0
        sel = (w >= w0) & (w < w1)
        base[sel] = cbase + (m[sel] * nw + (w[sel] - w0)) * 128
        cbase += nw * NCORES * 128
    return base + j


def host_prep(edges):
    """Route + sort edges, build the uniform chunk grid and slot arrays."""
    npc, win, node_pad, trows = _derived()
    src = np.asarray(edges[0], np.int64)
    dst = np.asarray(edges[1], np.int64)
    e = src.shape[0]

    core = dst // npc
    dl = dst - core * npc
    wi = dl >> 7
    dj = dl & 127
    prow = _prow_of(src)
    half = (prow >= HALF_SPLIT).astype(np.int64)

    # group key: (core, half, wi); lo region first per core.
    # within each group, slots sorted by source row (HBM locality).
    gkey = (core * 2 + half) * win + wi
    order = np.lexsort((prow, gkey))
    cnt = np.bincount(gkey, minlength=NCORES * 2 * win)
    goff = np.zeros_like(cnt)
    goff[1:] = np.cumsum(cnt)[:-1]
    rank = np.arange(e) - goff[gkey[order]]         # rank within group

    cnt3 = cnt.reshape(NCORES, 2, win)
    ncl = (cnt3[:, 0, :].max(axis=0) + 127) // 128  # lo chunks per window
    nch = (cnt3[:, 1, :].max(axis=0) + 127) // 128  # hi chunks per window
    ncl = np.maximum(ncl, 1)
    nch = np.maximum(nch, 1)
    nchl_tot = int(ncl.sum())
    nchh_tot = int(nch.sum())
    ncht = nchl_tot + nchh_tot
    lo_off = np.zeros(win, np.int64)
    lo_off[1:] = np.cumsum(ncl)[:-1]
    hi_off = np.zeros(win, np.int64)
    hi_off[1:] = np.cumsum(nch)[:-1]
    hi_off += nchl_tot

    # slot index for each (sorted) edge
    so = order
    chunk_base = np.where(half[so] == 0, lo_off[wi[so]], hi_off[wi[so]])
    slot = chunk_base * 128 + rank

    nslots = ncht * 128
    idx16 = np.zeros((NCORES, nslots), np.int16)
    pseudo_slot = np.zeros((NCORES, nslots, D), BF16)
    ivd_slot = np.zeros((NCORES, nslots), BF16)
    cs = core[so]
    iv = prow[so] - half[so] * HALF_SPLIT
    idx16[cs, slot] = iv.astype(np.int16)

    deg = np.bincount(dst, minlength=N).astype(np.float32)
    invdeg_flat = 1.0 / np.maximum(deg, 1.0)
    ivd_slot[cs, slot] = invdeg_flat[dst[so]].astype(BF16)

    # one-hot scatter matrices (pure 1.0 -> exact in fp8),
    # laid out [128 part, ncht, 128]  (slot s -> partition s%128, chunk s//128)
    S_tab = np.zeros((NCORES, nslots, 128), FP8)
    S_tab[cs, slot, dj[so]] = FP8(1.0)
    S_tab = S_tab.reshape(NCORES, ncht, 128, 128).transpose(0, 2, 1, 3).copy()

    return dict(
        order=order, slot=slot, core_sorted=cs,
        ncl=ncl, nch=nch, ncht=ncht, nchl_tot=nchl_tot,
        lo_off=lo_off, hi_off=hi_off,
        idx16=idx16, pseudo_slot=pseudo_slot,
        ivd_slot=ivd_slot, S_tab=S_tab,
        npc=npc, win=win, node_pad=node_pad, trows=trows,
    )


def fill_pseudo(prep, pseudo):
    ps = np.asarray(pseudo, np.float32)
    prep["pseudo_slot"][:] = 0.0
    prep["pseudo_slot"][prep["core_sorted"], prep["slot"]] = ps[prep["order"]].astype(BF16)


def build_program(prep):
    import concourse.bacc as bacc
    import concourse.mybir as mybir
    import concourse.tile as tile
    from concourse.library_config import mlp

    f32 = mybir.dt.float32
    bf16 = mybir.dt.bfloat16
    fp8 = mybir.dt.float8e4
    i16 = mybir.dt.int16
    AF = mybir.ActivationFunctionType
    OP = mybir.AluOpType

    win = prep["win"]
    node_pad = prep["node_pad"]
    trows = prep["trows"]
    ncht = prep["ncht"]
    ncl = prep["ncl"]
    nch = prep["nch"]
    lo_off = prep["lo_off"]
    hi_off = prep["hi_off"]
    nfull = float(N)
    agch = _ag_chunks(win)
    ag_last_w0, ag_last_w1 = agch[-1]

    # super-groups of windows
    sgs = [list(range(s, min(s + SG_WINDOWS, win))) for s in range(0, win, SG_WINDOWS)]
    n_sg = len(sgs)

    def sg_meta(sgi):
        sg_wins = sgs[sgi]
        w0, wE = sg_wins[0], sg_wins[-1]
        nlo = int(ncl[w0:wE + 1].sum())
        nhi = int(nch[w0:wE + 1].sum())
        return sg_wins, nlo, nhi, int(lo_off[w0]), int(hi_off[w0])

    nc = bacc.Bacc("TRN2", target_bir_lowering=False, num_devices=NCORES,
                   num_swdge_queues=4)

    def inp(name, shape, dt=f32):
        return nc.dram_tensor(name, shape, dt, kind="ExternalInput").ap()

    in_tab = inp("tab", [trows, 128], bf16)
    in_idx = inp("idx", [128, ncht * 8], i16)
    in_ps = inp("pseudo", [128, D, ncht], bf16)
    in_ivd = inp("ivdt", [128, ncht], bf16)
    in_S = inp("smat", [128, ncht, 128], fp8)
    in_xT = inp("xT0", [C, node_pad], bf16)
    in_ident = inp("ident", [128, 128], bf16)
    in_ones = inp("onesv", [128, 2], bf16)   # col0: ones, col1: valid mask last window
    in_zero = inp("zerov", [128, 1], bf16)
    in_G = [inp(f"g{l}c", [128, 2, C], bf16) for l in range(2)]
    in_RD = [inp(f"rd{l}", [C, C], bf16) for l in range(2)]
    in_MU = [inp(f"mu{l}t", [128, D, K], bf16) for l in range(2)]
    in_A = [inp(f"a{l}t", [128, D, K], bf16) for l in range(2)]
    in_gm = [inp(f"gamma{l}", [C, 1]) for l in range(2)]
    in_bt = [inp(f"beta{l}", [C, 1]) for l in range(2)]
    out_h = nc.dram_tensor("out", [node_pad, C], bf16, kind="ExternalOutput").ap()

    with tile.TileContext(nc) as tc, \
         nc.allow_low_precision(reason="bf16 pipeline; gate is 2e-2 rel"):
        nc.gpsimd.load_library(mlp)
        with tc.tile_pool(name="const", bufs=1) as cpool, \
             tc.tile_pool(name="qd", bufs=1) as qpool, \
             tc.tile_pool(name="sg", bufs=2) as sgp, \
             tc.tile_pool(name="wn", bufs=2) as wnp, \
             tc.tile_pool(name="per", bufs=1) as per, \
             tc.tile_pool(name="pB", bufs=2, space="PSUM") as pBp, \
             tc.tile_pool(name="pT", bufs=2, space="PSUM") as pTp, \
             tc.tile_pool(name="pH", bufs=2, space="PSUM") as pHp, \
             tc.tile_pool(name="pS", bufs=1, space="PSUM") as pSp, \
             tc.tile_pool(name="dram", bufs=1, space="DRAM") as dram:

            ident = cpool.tile([128, 128], bf16)
            nc.sync.dma_start(ident[:], in_ident[:])
            ones = cpool.tile([128, 2], bf16)
            nc.sync.dma_start(ones[:], in_ones[:])
            zero_t = cpool.tile([128, 1], bf16)
            nc.sync.dma_start(zero_t[:], in_zero[:])
            idx_t = cpool.tile([128, ncht * 8], i16)
            nc.sync.dma_start(idx_t[:], in_idx[:])
            ps_all = cpool.tile([128, D, ncht], bf16)
            nc.sync.dma_start(ps_all[:], in_ps[:])
            ivd_t = cpool.tile([128, ncht], bf16)
            nc.sync.dma_start(ivd_t[:], in_ivd[:])

            G_sb, RD_sb, gm_sb, bt_sb, quad = [], [], [], [], []
            for l in range(2):
                g_t = cpool.tile([128, 2, C], bf16, tag=f"G{l}")
                nc.sync.dma_start(g_t[:], in_G[l][:])
                G_sb.append(g_t)
                rd_t = cpool.tile([C, C], bf16, tag=f"RD{l}")
                nc.sync.dma_start(rd_t[:], in_RD[l][:])
                RD_sb.append(rd_t)
                gm_t = cpool.tile([C, 1], f32, tag=f"gm{l}")
                nc.sync.dma_start(gm_t[:], in_gm[l][:])
                gm_sb.append(gm_t)
                bt_t = cpool.tile([C, 1], f32, tag=f"bt{l}")
                nc.sync.dma_start(bt_t[:], in_bt[l][:])
                bt_sb.append(bt_t)
                mu_t = cpool.tile([128, D, K], bf16, tag=f"mu{l}")
                nc.sync.dma_start(mu_t[:], in_MU[l][:])
                a_t = cpool.tile([128, D, K], bf16, tag=f"a{l}")
                nc.sync.dma_start(a_t[:], in_A[l][:])
                quad.append((mu_t, a_t))

            # GMM weights w[p, k, c] = exp(-sum_d a_kd (p_d - mu_kd)^2)/deg,
            # computed in slot-major transposed layout with big contiguous ops
            def emit_quad(l):
                mu_t, a_t = quad[l]
                w_l = cpool.tile([128, K, ncht], bf16, tag="wl")
                dif = qpool.tile([128, D, K, ncht], bf16, tag="dif")
                nc.vector.tensor_tensor(
                    dif[:],
                    ps_all[:].unsqueeze(2).broadcast_to((128, D, K, ncht)),
                    mu_t[:].unsqueeze(3).broadcast_to((128, D, K, ncht)),
                    OP.subtract)
                nc.vector.tensor_tensor(dif[:], dif[:], dif[:], OP.mult)
                nc.vector.tensor_tensor(
                    dif[:], dif[:],
                    a_t[:].unsqueeze(3).broadcast_to((128, D, K, ncht)),
                    OP.mult)
                z = qpool.tile([128, K, ncht], bf16, tag="zq")
                nc.vector.tensor_tensor(z[:], dif[:, 0], dif[:, 1], OP.add)
                nc.vector.tensor_tensor(z[:], z[:], dif[:, 2], OP.add)
                nc.scalar.activation(w_l[:], z[:], AF.Exp, scale=-1.0)
                # fold 1/deg(dst) (zero on padding slots -> kills pad garbage)
                nc.vector.tensor_tensor(
                    w_l[:], w_l[:],
                    ivd_t[:].unsqueeze(1).broadcast_to((128, K, ncht)),
                    OP.mult)
                return w_l

            w_all = [emit_quad(0), None]

            tab1 = dram.tile([trows, 128], bf16)
            ag_in = dram.tile([node_pad, 128], bf16)
            ar_in = [dram.tile([C, 2], f32, tag=f"ari{l}", name=f"ari{l}") for l in range(2)]
            ar_out = [dram.tile([C, 2], f32, tag=f"aro{l}", name=f"aro{l}") for l in range(2)]

            def gather_views(tab_ap):
                return (tab_ap[0:HALF_SPLIT, :], tab_ap[HALF_SPLIT:trows, :])

            def emit_ag_chunk(w0, w1, h_sb):
                ag_view = ag_in.opt()[w0 * 128:w1 * 128, 0:C].rearrange(
                    "(w p) c -> p w c", p=128)
                nc.sync.dma_start(ag_view, h_sb[:, w0:w1, :])
                nc.gpsimd.collective_compute(
                    "AllGather", OP.bypass,
                    replica_groups=[list(range(NCORES))],
                    ins=[ag_in.opt()[w0 * 128:w1 * 128, :]],
                    outs=[tab1.opt()[w0 * NCORES * 128:w1 * NCORES * 128, :]])

            shr_mat = [None]

            def bcast_row(src_col, name):
                """[C,1] f32 column -> [128, C] bf16 broadcast-row tile."""
                b = per.tile([C, 1], bf16, tag=f"{name}b", name=f"{name}b")
                nc.vector.tensor_copy(b[:], src_col)
                pR = pTp.tile([1, C], bf16, tag="pT", name=f"{name}p")
                nc.tensor.transpose(pR[:], b[:], ident[0:C, 0:C])
                row = per.tile([1, C], bf16, tag=f"{name}r", name=f"{name}r")
                nc.vector.tensor_copy(row[:], pR[:])
                mat = per.tile([128, C], bf16, tag=f"{name}m", name=f"{name}m")
                nc.gpsimd.partition_broadcast(mat[:], row[:])
                return mat

            hT_prev = None
            for layer in range(2):
                w_l = w_all[layer]
                h_sb = per.tile([128, win, C], bf16, tag="h", name=f"h{layer}")
                hTn = (per.tile([C, node_pad], bf16, tag="hT0", name="hT0")
                       if layer == 0 else None)
                pstat = pSp.tile([C, 1], f32, tag="st")
                pstat2 = pSp.tile([C, 1], f32, tag="st2")

                tviews = gather_views(in_tab if layer == 0 else tab1.opt())
                gq = [0]
                for sgi in range(n_sg):
                    sg_wins, nlo, nhi, clo0, chi0 = sg_meta(sgi)
                    nsg = nlo + nhi
                    xg = sgp.tile([128, nsg, 128], bf16, tag="xg", bufs=8)
                    for (base, n, goff, tview) in (
                            (0, nlo, clo0, tviews[0]),
                            (nlo, nhi, chi0, tviews[1])):
                        for g0 in range(0, n, GMAX):
                            gn = min(GMAX, n - g0)
                            c0 = base + g0
                            gc = goff + g0
                            nc.gpsimd.dma_gather(
                                xg[:, c0:c0 + gn, :], tview,
                                idx_t[:, gc * 8:(gc + gn) * 8],
                                gn * 128, gn * 128, 128,
                                single_packet=(gn <= 8),
                                queue_num=gq[0] % 4)
                            gq[0] += 1

                    if layer == 1:
                        # BN0 shift + ReLU on the gathered pre-BN rows
                        # (BN0 scale is folded into G1/RD1)
                        xgv = xg[:, :, 0:C]
                        nc.vector.tensor_tensor(
                            xgv, xgv,
                            shr_mat[0][:].unsqueeze(1).broadcast_to((128, nsg, C)),
                            OP.add)
                        nc.vector.tensor_tensor(
                            xgv, xgv,
                            zero_t[:].unsqueeze(1).broadcast_to((128, nsg, C)),
                            OP.max)

                    S_t = sgp.tile([128, nsg, 128], fp8, tag="smat")
                    nc.sync.dma_start(S_t[:, :nlo, :], in_S[:, clo0:clo0 + nlo, :])
                    nc.sync.dma_start(S_t[:, nlo:, :], in_S[:, chi0:chi0 + nhi, :])

                    # xw = xg * w, batched over the whole super-group per half
                    xw = wnp.tile([128, nsg, K, C], bf16, tag="xw")
                    for (o, n, gc0) in ((0, nlo, clo0), (nlo, nhi, chi0)):
                        nc.vector.tensor_tensor(
                            xw[:, o:o + n, :, :],
                            xg[:, o:o + n, 0:C].unsqueeze(2)
                                .broadcast_to((128, n, K, C)),
                            w_l[:, :, gc0:gc0 + n].transpose([0, 2, 1])
                                .unsqueeze(3).broadcast_to((128, n, K, C)),
                            OP.mult)

                    lo_c = 0
                    hi_c = nlo
                    for wi_ in sg_wins:
                        nl = int(ncl[wi_])
                        nh = int(nch[wi_])
                        ncw = nl + nh
                        ranges = [(lo_c, nl), (hi_c, nh)]

                        pB = pBp.tile([128, K * C], f32, tag="pB")
                        cj = 0
                        for (c0, n) in ranges:
                            for j in range(n):
                                nc.tensor.matmul(
                                    pB[:], S_t[:, c0 + j, :], xw[:, c0 + j, :, :].opt(),
                                    start=(cj == 0), stop=(cj == ncw - 1))
                                cj += 1

                        bsb = wnp.tile([128, K * C], bf16, tag="bsb")
                        nc.scalar.activation(bsb[:], pB[:], AF.Copy)

                        pT = pTp.tile([128, K * C], bf16, tag="pT")
                        nc.tensor.transpose(pT[:, 0:128], bsb[:, 0:128], ident[:])
                        nc.tensor.transpose(pT[:, 128:256], bsb[:, 128:256], ident[:])
                        bT = wnp.tile([128, K * C], bf16, tag="bT")
                        nc.scalar.activation(bT[:], pT[:], AF.Copy)

                        if layer == 0:
                            xTw = wnp.tile([C, 128], bf16, tag="xTw")
                            nc.sync.dma_start(
                                xTw[:], in_xT[:, wi_ * 128:(wi_ + 1) * 128])
                            xT_ap = xTw[:]
                        else:
                            xT_ap = hT_prev[:, wi_ * 128:(wi_ + 1) * 128]

                        pH = pHp.tile([128, C], f32, tag="pH")
                        nc.tensor.matmul(pH[:], bT[:, 0:128], G_sb[layer][:, 0, :],
                                         start=True, stop=False)
                        nc.tensor.matmul(pH[:], bT[:, 128:256], G_sb[layer][:, 1, :],
                                         start=False, stop=False)
                        nc.tensor.matmul(pH[:], xT_ap, RD_sb[layer][:],
                                         start=False, stop=True)

                        nc.scalar.activation(h_sb[:, wi_, :], pH[:], AF.Copy)
                        hsq = wnp.tile([128, C], bf16, tag="hsq")
                        nc.scalar.activation(hsq[:], h_sb[:, wi_, :], AF.Square)
                        mcol = 1 if wi_ == win - 1 else 0
                        nc.tensor.matmul(pstat[:], h_sb[:, wi_, :],
                                         ones[:, mcol:mcol + 1],
                                         start=(wi_ == 0), stop=(wi_ == win - 1),
                                         skip_group_check=True)
                        nc.tensor.matmul(pstat2[:], hsq[:],
                                         ones[:, mcol:mcol + 1],
                                         start=(wi_ == 0), stop=(wi_ == win - 1),
                                         skip_group_check=True)

                        if layer == 0:
                            # pre-BN transposed copy (BN applied in-place later)
                            pT2 = pTp.tile([C, 128], bf16, tag="pT")
                            nc.tensor.transpose(pT2[:], h_sb[:, wi_, :], ident[:])
                            nc.scalar.activation(hTn[:, wi_ * 128:(wi_ + 1) * 128],
                                                 pT2[:], AF.Copy)

                        # stream completed AllGather chunks during layer 0
                        if layer == 0 and wi_ + 1 in [c[1] for c in agch[:-1]]:
                            w0 = [c[0] for c in agch if c[1] == wi_ + 1][0]
                            emit_ag_chunk(w0, wi_ + 1, h_sb)

                        lo_c += nl
                        hi_c += nh

                # BN stats all-reduce
                st = per.tile([C, 2], f32, tag=f"stsb{layer}")
                nc.vector.tensor_copy(st[:, 0:1], pstat[:])
                nc.vector.tensor_copy(st[:, 1:2], pstat2[:])
                nc.sync.dma_start(ar_in[layer][:], st[:])
                nc.gpsimd.collective_compute(
                    "AllReduce", OP.add,
                    replica_groups=[list(range(NCORES))],
                    ins=[ar_in[layer].opt()], outs=[ar_out[layer].opt()])
                if layer == 0:
                    # final AllGather chunk
                    emit_ag_chunk(ag_last_w0, ag_last_w1, h_sb)
                    # layer-1 GMM weights: fills the otherwise-idle DVE
                    w_all[1] = emit_quad(1)
                stg = per.tile([C, 2], f32, tag=f"stg{layer}")
                nc.sync.dma_start(stg[:], ar_out[layer][:])

                mean = per.tile([C, 1], f32, tag=f"mean{layer}")
                nc.vector.tensor_scalar(mean[:], stg[:, 0:1], 1.0 / nfull, None, OP.mult)
                var = per.tile([C, 1], f32, tag=f"var{layer}")
                nc.vector.tensor_scalar(var[:], stg[:, 1:2], 1.0 / nfull, None, OP.mult)
                msq = per.tile([C, 1], f32, tag=f"msq{layer}")
                nc.vector.tensor_tensor(msq[:], mean[:], mean[:], OP.mult)
                nc.vector.tensor_tensor(var[:], var[:], msq[:], OP.subtract)
                nc.vector.tensor_scalar(var[:], var[:], BN_EPS, None, OP.add)
                sd = per.tile([C, 1], f32, tag=f"sd{layer}")
                nc.scalar.activation(sd[:], var[:], AF.Sqrt)
                rstd = per.tile([C, 1], f32, tag=f"rstd{layer}")
                nc.vector.reciprocal(rstd[:], sd[:])
                scl = per.tile([C, 1], f32, tag=f"scl{layer}")
                nc.vector.tensor_tensor(scl[:], gm_sb[layer][:], rstd[:], OP.mult)
                sh = per.tile([C, 1], f32, tag=f"sh{layer}")
                nc.vector.tensor_tensor(sh[:], mean[:], scl[:], OP.mult)
                nc.vector.tensor_tensor(sh[:], bt_sb[layer][:], sh[:], OP.subtract)

                if layer == 0:
                    # shr = sh/scl; local transposed copy gets x~ + shr, ReLU
                    # (per-partition bias works in the transposed domain);
                    # the scale scl is folded into G1 and RD1 rows.
                    rscl = per.tile([C, 1], f32, tag="rscl")
                    nc.vector.reciprocal(rscl[:], scl[:])
                    shr = per.tile([C, 1], f32, tag="shr")
                    nc.vector.tensor_tensor(shr[:], sh[:], rscl[:], OP.mult)
                    nc.scalar.activation(hTn[:], hTn[:], AF.Relu, bias=shr[:])

                    # shr as a broadcast row for the gathered-tile BN
                    shr_mat[0] = bcast_row(shr[:], "shr")

                    # scl as [128,1] pattern scl[p%64] for the G1 row fold
                    scl_b = per.tile([C, 1], bf16, tag="sclb")
                    nc.vector.tensor_copy(scl_b[:], scl[:])
                    pRowA = pTp.tile([1, C], bf16, tag="pT")
                    nc.tensor.transpose(pRowA[:], scl_b[:], ident[0:C, 0:C])
                    rowAA = per.tile([1, 128], bf16, tag="rowAA")
                    nc.vector.tensor_copy(rowAA[:, 0:C], pRowA[:])
                    nc.vector.tensor_copy(rowAA[:, C:128], pRowA[:])
                    pCol = pTp.tile([128, 1], bf16, tag="pT")
                    nc.tensor.transpose(pCol[:], rowAA[:], ident[0:1, 0:1])
                    sclpat = per.tile([128, 1], f32, tag="sclpat")
                    nc.vector.tensor_copy(sclpat[:], pCol[:])
                    nc.scalar.activation(G_sb[1][:], G_sb[1][:], AF.Copy,
                                         scale=sclpat[:])
                    nc.scalar.activation(RD_sb[1][:], RD_sb[1][:], AF.Copy,
                                         scale=scl[:])
                    hT_prev = hTn
                else:
                    # BN applied directly in the node-major domain (h_sb) —
                    # no transposed copy or transpose-back needed
                    scl1_mat = bcast_row(scl[:], "scl1")
                    sh1_mat = bcast_row(sh[:], "sh1")
                    hn = per.tile([128, win, C], bf16, tag="hn", name=f"hn{layer}")
                    nc.vector.tensor_tensor(
                        hn[:], h_sb[:],
                        scl1_mat[:].unsqueeze(1).broadcast_to((128, win, C)),
                        OP.mult)
                    nc.vector.tensor_tensor(
                        hn[:], hn[:],
                        sh1_mat[:].unsqueeze(1).broadcast_to((128, win, C)),
                        OP.add)
                    out_view = out_h.rearrange("(w p) c -> p w c", p=128)
                    nc.sync.dma_start(out_view, hn[:])

    nc.compile()
    return nc


def make_in_maps(prep, inputs):
    npc, win, node_pad, trows = _derived()
    vals = np.asarray(inputs["vals"], np.float32)
    ident = np.eye(128, dtype=BF16)

    node_prow = _prow_of(np.arange(N, dtype=np.int64))
    tab = np.zeros((trows, 128), BF16)
    tab[node_prow, 0:C] = vals.astype(BF16)

    ncht = prep["ncht"]
    onesv = np.zeros((128, 2), BF16)
    onesv[:, 0] = 1.0
    tail = npc - (win - 1) * 128
    onesv[:tail, 1] = 1.0

    shared = {"ident": ident, "onesv": onesv, "tab": tab,
              "zerov": np.zeros((128, 1), BF16)}
    for l in range(2):
        g = np.asarray(inputs[f"g{l}"], np.float32)          # [C, K*C]
        G = np.zeros((K * C, C), np.float32)                 # G[k*C+c, c'] = g[c, k*C+c']
        for k in range(K):
            G[k * C:(k + 1) * C, :] = g[:, k * C:(k + 1) * C]
        shared[f"g{l}c"] = G.reshape(2, 128, C).transpose(1, 0, 2).astype(BF16)
        shared[f"rd{l}"] = (np.asarray(inputs[f"root{l}"], np.float32)
                            + np.asarray(inputs[f"dense{l}"], np.float32)).astype(BF16)
        mu = np.asarray(inputs[f"mu{l}"], np.float64)        # [K, D]
        sg = np.asarray(inputs[f"sigma{l}"], np.float64)
        a = 0.5 / (EPS + sg * sg)                            # [K, D]
        shared[f"mu{l}t"] = np.broadcast_to(
            mu.T[None], (128, D, K)).astype(BF16)
        shared[f"a{l}t"] = np.broadcast_to(
            a.T[None], (128, D, K)).astype(BF16)
        shared[f"gamma{l}"] = np.asarray(inputs[f"gamma{l}"], np.float32).reshape(C, 1)
        shared[f"beta{l}"] = np.asarray(inputs[f"beta{l}"], np.float32).reshape(C, 1)

    in_maps = []
    s = np.arange(ncht * 128)
    for m in range(NCORES):
        nslots = ncht * 128
        blk = np.zeros((16, nslots // 16), np.int16)
        blk[s % 16, s // 16] = prep["idx16"][m]
        idx_w = np.tile(blk, (8, 1))

        ps = np.zeros((128, D, ncht), BF16)
        ps[s % 128, :, s // 128] = prep["pseudo_slot"][m]
        ivdt = np.zeros((128, ncht), BF16)
        ivdt[s % 128, s // 128] = prep["ivd_slot"][m]

        xT0 = np.zeros((C, node_pad), BF16)
        xT0[:, :npc] = vals[m * npc:(m + 1) * npc].T

        in_maps.append(dict(shared, idx=idx_w, pseudo=ps, ivdt=ivdt,
                            smat=prep["S_tab"][m], xT0=xT0))
    return in_maps


_CACHE = {}


def kernel(**inputs):
    global LAST_RESULT
    from concourse.bass_utils import run_bass_kernel_spmd

    npc, win, node_pad, trows = _derived()
    edges = np.asarray(inputs["edges"])
    ekey = hash(edges.tobytes())
    if ekey in _CACHE:
        prep, nc = _CACHE[ekey]
    else:
        prep = host_prep(edges)
        nc = None
    fill_pseudo(prep, inputs["pseudo"])
    if nc is None:
        nc = build_program(prep)
        _CACHE[ekey] = (prep, nc)
    in_maps = make_in_maps(prep, inputs)
    trace = bool(os.environ.get("BASS_KERNEL_TRACE"))
    import time as _time
    _t0 = _time.time()
    res = run_bass_kernel_spmd(nc, in_maps, list(range(NCORES)), trace=trace)
    print(f"[kernel] run_bass_kernel_spmd wall: {_time.time() - _t0:.2f}s", file=sys.stderr)
    LAST_RESULT = res
    out = np.concatenate(
        [res.results[m]["out"][:npc] for m in range(NCORES)], axis=0)
    return np.ascontiguousarray(out, dtype=np.float32)
